# revision 23
# baseline (speedup 1.0000x reference)
"""Trainium2 Bass kernel: AggregateEdgesFromNodes (GNN message passing).

h = relu(node_edge_feat[srcs] @ W[:128]
         + node_edge_feat[dsts] @ W[128:256]
         + dist_feat @ W[256:384] + b)

Strategy
--------
Edges are distributed over the 8 NeuronCores; the node/edge feature table and
the 384x128 weight are replicated. The per-edge row gather runs on-device via
the GPSIMD vector-gather instruction (`dma_gather`), which takes int16 row
offsets relative to a per-instruction window base. To cover the 850k-row table
with int16 offsets, edges are grouped by the (src-window, dst-window) pair —
13 signed windows of 65534 rows each, 169 groups — so each group's src and dst
gathers each address a single window. Groups are padded to a fixed 640-edge
capacity and split round-robin across cores, so one static program serves all
8 cores (SPMD). Gathered tiles are transposed on the PE (feature dim onto
partitions), three accumulating f32r matmuls with the W blocks stationary run
per 320-edge subtile, and bias+relu lands on the scalar engine. dist_feat is
fed and the output returned in feature-major, group-sorted order; the host
permutes both. Edges whose in-window offset is exactly -1 (the gather's
"invalid" sentinel; ~1 per million) are recomputed on the host and patched.
"""

import os

from contextlib import ExitStack

import numpy as np

import concourse.bass as bass
import concourse.mybir as mybir
import concourse.tile as tile
from concourse import bacc
from concourse.bass_utils import run_bass_kernel_spmd
from concourse.masks import make_identity

N_CORES = 8
NUM_NODES = 850000
NUM_EDGES = 800000
HIDDEN = 128
P = 128

BIN_W = 65534                    # signed-int16 addressable window
N_BINS = -(-NUM_NODES // BIN_W)  # 13
N_GROUPS = N_BINS * N_BINS       # 169
CAP = 640                        # edges per (group, core); 5 blocks of 128
CAP_BLK = CAP // P               # 5
SUB = 320                        # GEMM subtile (fits one PSUM bank, N>=256)
E2 = N_GROUPS * CAP              # 108160 padded edges per core

f32 = mybir.dt.float32
f32r = mybir.dt.float32r
i16 = mybir.dt.int16

LAST_RESULTS = None


def _center(b):
    return b * BIN_W + 32768


def build_kernel(cap=CAP, num_devices=N_CORES):
    cap_blk = cap // P
    scols = cap // 16            # idx columns per group
    e2 = N_GROUPS * cap

    nc = bacc.Bacc("TRN2", target_bir_lowering=False, debug=False,
                   enable_asserts=False, num_devices=num_devices,
                   num_swdge_queues=4, dynamic_dma_scratch_size=131072)
    table = nc.dram_tensor("table", [NUM_NODES, HIDDEN], f32,
                           kind="ExternalInput")
    distT = nc.dram_tensor("distT", [HIDDEN, e2], f32, kind="ExternalInput")
    sidx_d = nc.dram_tensor("sidx", [P, e2 // 16], i16, kind="ExternalInput")
    didx_d = nc.dram_tensor("didx", [P, e2 // 16], i16, kind="ExternalInput")
    w_d = nc.dram_tensor("w", [3 * HIDDEN, HIDDEN], f32, kind="ExternalInput")
    b_d = nc.dram_tensor("b", [HIDDEN, 1], f32, kind="ExternalInput")
    outT = nc.dram_tensor("outT", [HIDDEN, e2], f32, kind="ExternalOutput")

    with tile.TileContext(nc) as tc, ExitStack() as ctx:
        const = ctx.enter_context(tc.tile_pool(name="const", bufs=1))
        gpool = ctx.enter_context(tc.tile_pool(name="gather", bufs=3))
        spool = ctx.enter_context(tc.tile_pool(name="sbufw", bufs=3))
        opool = ctx.enter_context(tc.tile_pool(name="outp", bufs=3))
        psum = ctx.enter_context(tc.tile_pool(name="psum", bufs=2,
                                              space="PSUM"))

        ident = const.tile([P, P], f32)
        make_identity(nc, ident[:])
        ws = []
        for sblk in range(3):
            wt = const.tile([P, HIDDEN], f32r, tag=f"w{sblk}", name=f"w{sblk}")
            nc.sync.dma_start(out=wt[:],
                              in_=w_d[sblk * HIDDEN:(sblk + 1) * HIDDEN,
                                      :].bitcast(f32r))
            ws.append(wt)
        bt = const.tile([P, 1], f32)
        nc.sync.dma_start(out=bt[:], in_=b_d[:, :])
        sidx = const.tile([P, e2 // 16], i16, tag="sidx", name="sidx")
        nc.sync.dma_start(out=sidx[:], in_=sidx_d[:, :])
        didx = const.tile([P, e2 // 16], i16, tag="didx", name="didx")
        nc.sync.dma_start(out=didx[:], in_=didx_d[:, :])

        SRCG = 1
        for bs in range(N_BINS):
            gs = None
            for bd in range(N_BINS):
                g = bs * N_BINS + bd
                c0 = g * scols
                off = g * cap
                if bd % SRCG == 0:
                    ng = min(SRCG, N_BINS - bd)
                    gs = gpool.tile([P, ng * cap_blk, HIDDEN], f32,
                                    tag="gs", name="gs")
                    nc.gpsimd.dma_gather(
                        out_ap=gs[:, :, :],
                        in_ap=table[_center(bs):_center(bs) + 2, :],
                        idxs_ap=sidx[:, c0:c0 + ng * scols],
                        num_idxs=ng * cap, num_idxs_reg=ng * cap,
                        elem_size=HIDDEN, single_packet=False,
                        queue_num=(2 * g) % 4)
                gd = gpool.tile([P, cap_blk, HIDDEN], f32, tag="gd", name="gd")
                nc.gpsimd.dma_gather(
                    out_ap=gd[:, :, :],
                    in_ap=table[_center(bd):_center(bd) + 2, :],
                    idxs_ap=didx[:, c0:c0 + scols],
                    num_idxs=cap, num_idxs_reg=cap, elem_size=HIDDEN,
                    single_packet=False, queue_num=(2 * g + 1) % 4)

                xst_ps = psum.tile([P, cap], f32, tag="tps", name="xst_ps")
                for j in range(cap_blk):
                    nc.tensor.transpose(
                        out=xst_ps[:, j * P:(j + 1) * P],
                        in_=gs[:, (bd % SRCG) * cap_blk + j, :],
                        identity=ident[:])
                xst = spool.tile([P, cap], f32r, tag="xst", name="xst")
                nc.vector.tensor_copy(out=xst[:], in_=xst_ps[:])

                xdt_ps = psum.tile([P, cap], f32, tag="tps", name="xdt_ps")
                for j in range(cap_blk):
                    nc.tensor.transpose(out=xdt_ps[:, j * P:(j + 1) * P],
                                        in_=gd[:, j, :],
                                        identity=ident[:])
                xdt = spool.tile([P, cap], f32r, tag="xdt", name="xdt")
                nc.vector.tensor_copy(out=xdt[:], in_=xdt_ps[:])

                xdist = spool.tile([P, cap], f32r, tag="xdist", name="xdist")
                nc.sync.dma_start(out=xdist[:],
                                  in_=distT[:, off:off + cap].bitcast(f32r))

                for s in range(cap // SUB):
                    sl = slice(s * SUB, (s + 1) * SUB)
                    h_ps = psum.tile([P, SUB], f32, tag="h", name="h_ps",
                                     bufs=4)
                    nc.tensor.matmul(out=h_ps[:], lhsT=ws[0][:],
                                     rhs=xst[:, sl], start=True, stop=False)
                    nc.tensor.matmul(out=h_ps[:], lhsT=ws[1][:],
                                     rhs=xdt[:, sl], start=False, stop=False)
                    nc.tensor.matmul(out=h_ps[:], lhsT=ws[2][:],
                                     rhs=xdist[:, sl], start=False, stop=True)
                    o = opool.tile([P, SUB], f32, tag="o", name="o")
                    nc.scalar.activation(
                        out=o[:], in_=h_ps[:],
                        func=mybir.ActivationFunctionType.Relu, bias=bt[:])
                    nc.sync.dma_start(
                        out=outT[:, off + s * SUB:off + (s + 1) * SUB],
                        in_=o[:])
    nc.compile()
    return nc


_COMPILED = {}


def _get_compiled(cap):
    if cap not in _COMPILED:
        _COMPILED[cap] = build_kernel(cap=cap)
    return _COMPILED[cap]


def _pack_idx16(stream):
    """int16 stream -> [128, len/16]: position i -> (partition i%16, col i//16),
    replicated across the 8 partition groups."""
    s16 = len(stream) // 16
    base = stream.reshape(s16, 16).T
    return np.ascontiguousarray(np.tile(base, (8, 1)))


def _prepare(node_edge_feat, dist_feat, srcs, dsts, W, b):
    E = srcs.shape[0]
    sbin = srcs // BIN_W
    dbin = dsts // BIN_W
    slo = (srcs - sbin * BIN_W - 32768).astype(np.int64)   # in [-32768, 32765]
    dlo = (dsts - dbin * BIN_W - 32768).astype(np.int64)
    dead = (slo == -1) | (dlo == -1)
    slo = np.where(slo == -1, 0, slo).astype(np.int16)
    dlo = np.where(dlo == -1, 0, dlo).astype(np.int16)

    grp = (sbin * N_BINS + dbin).astype(np.int64)
    order = np.argsort(grp, kind="stable")
    grp_sorted = grp[order]
    counts = np.bincount(grp, minlength=N_GROUPS)
    starts = np.concatenate([[0], np.cumsum(counts)[:-1]])
    rank = np.arange(E) - starts[grp_sorted]
    core_of = (rank % N_CORES).astype(np.int64)
    slot_of = rank // N_CORES

    cap = CAP
    max_slot = int(slot_of.max()) if E else 0
    if max_slot >= cap:
        cap = -(-(max_slot + 1) // P) * P
    e2 = N_GROUPS * cap

    # stream position of each (sorted) edge within its core
    pos = grp_sorted * cap + slot_of

    in_maps = []
    orig_of_core = []
    for c in range(N_CORES):
        m = core_of == c
        p_c = pos[core_of == c]
        e_c = order[m]
        orig = np.full(e2, -1, np.int64)
        orig[p_c] = e_c
        orig_of_core.append(orig)

        s16 = np.zeros(e2, np.int16)
        d16 = np.zeros(e2, np.int16)
        s16[p_c] = slo[e_c]
        d16[p_c] = dlo[e_c]

        dist_pad = np.zeros((e2, HIDDEN), np.float32)
        valid = orig >= 0
        dist_pad[valid] = dist_feat[orig[valid]]

        in_maps.append({
            "table": node_edge_feat,
            "distT": np.ascontiguousarray(dist_pad.T),
            "sidx": _pack_idx16(s16),
            "didx": _pack_idx16(d16),
            "w": W,
            "b": b.reshape(HIDDEN, 1),
        })

    return in_maps, orig_of_core, cap, dead


def _finalize(out_ts, orig_of_core, dead, node_edge_feat, dist_feat,
              srcs, dsts, W, b):
    E = srcs.shape[0]
    out = np.empty((E, HIDDEN), np.float32)
    for c in range(N_CORES):
        orig = orig_of_core[c]
        valid = orig >= 0
        out[orig[valid]] = out_ts[c].T[valid]

    if dead.any():
        de = np.where(dead)[0]
        h = (node_edge_feat[srcs[de]] @ W[:HIDDEN]
             + node_edge_feat[dsts[de]] @ W[HIDDEN:2 * HIDDEN]
             + dist_feat[de] @ W[2 * HIDDEN:] + b)
        out[de] = np.maximum(h, 0.0)
    return out


def kernel(node_edge_feat, dist_feat, srcs, dsts, W, b):
    node_edge_feat = np.ascontiguousarray(np.asarray(node_edge_feat),
                                          dtype=np.float32)
    dist_feat = np.ascontiguousarray(np.asarray(dist_feat), dtype=np.float32)
    srcs = np.asarray(srcs).astype(np.int64)
    dsts = np.asarray(dsts).astype(np.int64)
    W = np.ascontiguousarray(np.asarray(W), dtype=np.float32)
    b = np.ascontiguousarray(np.asarray(b), dtype=np.float32)

    in_maps, orig_of_core, cap, dead = _prepare(
        node_edge_feat, dist_feat, srcs, dsts, W, b)
    nc = _get_compiled(cap)

    trace = bool(int(os.environ.get("KERNEL_TRACE", "0")))
    try:
        res = run_bass_kernel_spmd(nc, in_maps, list(range(N_CORES)),
                                   trace=trace)
    except Exception:
        if not trace:
            raise
        # tracing machinery unavailable; fall back to a plain run
        res = run_bass_kernel_spmd(nc, in_maps, list(range(N_CORES)),
                                   trace=False)
    global LAST_RESULTS
    LAST_RESULTS = res

    out_ts = [res.results[c]["outT"] for c in range(N_CORES)]
    return _finalize(out_ts, orig_of_core, dead, node_edge_feat, dist_feat,
                     srcs, dsts, W, b)
